# revision 38
# baseline (speedup 1.0000x reference)
"""Trainium2 Bass kernel for label-attention:
    scores = einsum('cd,bld->bcl', U, keys) / sqrt(D)
    alpha  = softmax(scores, axis=l)
    v      = einsum('bcl,bld->bcd', alpha, keys)

Sharding: data-parallel over batch across 8 NeuronCores (2 batches/core,
U replicated). No collectives; host gathers per-core outputs.

Per-core pipeline (all matmuls in bf16, fp32 accumulation):
  prep:  cast K -> bf16 "K_aug" = [K | ones] with l on partitions,
         build K^T and U^T (d on partitions) via DMA transpose.
  main:  for each (b, c-tile of 512):
           for each l-chunk of 128:
             S^T[l, c512] = sum_d K^T[d,l].T @ U^T[d,c]   (PE, 2 matmuls N=512)
             E = exp(S^T * 1/sqrt(D))                     (ACT, PSUM->SBUF bf16)
             for j in 4: pv[c128, 257] += E[:,j].T @ K_aug (PE, N=257)
           pv[:, :256] is the softmax numerator @ K; pv[:, 256] the denominator
           (ones column) -- softmax division is deferred to the epilogue.
           epilogue: v = pv[:, :256] * (1 / pv[:, 256]); DMA out.
  Max-subtraction is skipped: logits are (U@K^T)/16 with xavier-uniform U,
  |logit| < ~0.5, so exp() is numerically safe and softmax is algebraically
  identical to the max-subtracted form.
"""

import math
import os
import sys
from contextlib import ExitStack

import numpy as np

# concourse ships with the container; make sure it's importable.
for _p in ("/opt/trn_rl_repo", "/root/.axon_site/_ro/trn_rl_repo"):
    if _p not in sys.path and os.path.isdir(_p):
        sys.path.append(_p)

import concourse.bass as bass  # noqa: E402
import concourse.bacc as bacc  # noqa: E402
import concourse.mybir as mybir  # noqa: E402
import concourse.tile as tile  # noqa: E402

F32 = mybir.dt.float32
BF16 = mybir.dt.bfloat16
FP8 = mybir.dt.float8e4
P = 128

# fp8 pre-scales keep U/K values in e4m3's normal range; the product scale
# (U_SCALE * K_SCALE) is divided back out inside the exp activation.
U_SCALE = 256.0
K_SCALE = 4.0

# Problem shape (hardcoded per contest contract).
B_FULL = 16
L_FULL = 2048
D_FULL = 256
C_FULL = 5000
N_CORES = 8
B_LOC = B_FULL // N_CORES  # 2 batches per core


def _build_nc(
    B_loc=B_LOC,
    L=L_FULL,
    C=C_FULL,
    D=D_FULL,
    C_TILE=512,
    transpose_mode="pe",  # "sbuf" | "dram" | "pe" | "hybrid"
    mm1_fp8=True,  # fp8e4m3 DoubleRow for the scores matmul
):
    NL = L // P
    ND = D // P
    NCT = math.ceil(C / C_TILE)
    C_PAD = NCT * C_TILE
    CSUB = C_TILE // P
    assert NL % 2 == 0, "exp pairing assumes an even number of l-chunks"
    scale = 1.0 / math.sqrt(D)
    mm_dt = FP8 if mm1_fp8 else BF16
    if mm1_fp8:
        scale /= U_SCALE * K_SCALE

    nc = bacc.Bacc("TRN2", target_bir_lowering=False, debug=False)
    keys_d = nc.dram_tensor("keys", [B_loc, L, D], F32, kind="ExternalInput")
    u_d = nc.dram_tensor("U_weight", [C, D], F32, kind="ExternalInput")
    out_d = nc.dram_tensor("out", [B_loc, C, D], F32, kind="ExternalOutput")

    with tile.TileContext(nc) as tc, ExitStack() as ctx:
        const = ctx.enter_context(tc.tile_pool(name="const", bufs=1))
        persist = ctx.enter_context(tc.tile_pool(name="persist", bufs=1))
        stage = ctx.enter_context(tc.tile_pool(name="stage", bufs=8))
        expp = ctx.enter_context(tc.tile_pool(name="expp", bufs=3))
        outp = ctx.enter_context(tc.tile_pool(name="outp", bufs=6))

        prep_ctx = ExitStack()
        if transpose_mode in ("pe", "hybrid"):
            from concourse.masks import make_identity

            psT = prep_ctx.enter_context(tc.tile_pool(name="psT", bufs=2, space="PSUM"))
            ident = const.tile([P, P], BF16, tag="ident", name="ident")
            make_identity(nc, ident)
            identf = const.tile([P, P], F32, tag="identf", name="identf")
            make_identity(nc, identf)

        zbias = const.tile([P, 1], F32, tag="zbias", name="zbias")
        nc.gpsimd.memset(zbias[:], 0.0)

        # Persistent bf16 operands.
        # UT[d, c] (d on partitions, 2 chunks) / KT[b][d, l] / KA[b][l, d+ones]
        UT = persist.tile([P, ND, C_PAD], mm_dt, tag="UT", name="UT")
        KT = [
            persist.tile([P, ND, L], mm_dt, tag=f"KT{b}", name=f"KT{b}")
            for b in range(B_loc)
        ]
        KA = [
            persist.tile([P, NL, D + 1], BF16, tag=f"KA{b}", name=f"KA{b}")
            for b in range(B_loc)
        ]

        if transpose_mode == "dram":
            dram = ctx.enter_context(tc.tile_pool(name="dram", bufs=1, space="DRAM"))
            ubf_d = dram.tile([C_PAD, D], BF16, tag="ubf", name="ubf")
            kbf_d = dram.tile([B_loc, L, D], BF16, tag="kbf", name="kbf")

        def transpose_128(dst_ap, src_ap, scale_out=1.0):
            """dst[128, 128] = src[128, 128].T * scale_out (casts to dst dtype)."""
            if transpose_mode == "sbuf":
                nc.sync.dma_start_transpose(dst_ap, src_ap)
            else:  # "pe" / "hybrid"
                pt = psT.tile([P, P], BF16, tag="pt", name="pt")
                nc.tensor.transpose(pt[:], src_ap, ident[:])
                if scale_out == 1.0:
                    nc.vector.tensor_copy(dst_ap, pt[:])
                else:
                    nc.vector.tensor_scalar_mul(dst_ap, pt[:], scale_out)

        def prep_k(b):
            # loads -> GpSimd casts (frees the DVE); transposes grouped 4 per
            # PSUM bank so one DVE copy moves 512 columns.
            for n in range(NL):
                kst = stage.tile([P, D], F32, tag="kstage", name="kst")
                nc.sync.dma_start(kst[:], keys_d[b, n * P : (n + 1) * P, :])
                nc.gpsimd.tensor_copy(KA[b][:, n, 0:D], kst[:])
            nc.any.memset(KA[b][:, :, D : D + 1], 1.0)
            for dd in range(ND):
                for g in range(0, NL, 4):
                    pt = psT.tile([P, 4, P], BF16, tag="pt", name="pt")
                    for i in range(4):
                        nc.tensor.transpose(
                            pt[:, i, :],
                            KA[b][:, g + i, dd * P : (dd + 1) * P],
                            ident[:],
                        )
                    nc.vector.tensor_scalar_mul(
                        KT[b][:, dd, g * P : (g + 4) * P],
                        pt[:],
                        K_SCALE if mm1_fp8 else 1.0,
                    )

        def prep_u(ct):
            # f32 transposed directly on the PE, 4 per PSUM bank; the single
            # PSUM->SBUF copy per (ct, dd) runs on ScalarE (idle during prep)
            # with the fp8 scale+cast fused.
            usts = []
            for s in range(CSUB):
                r0 = (ct * CSUB + s) * P
                rows = min(P, C - r0)
                ust = stage.tile([P, D], F32, tag="ustage", name="ust")
                if rows < P:
                    nc.any.memset(ust[:], 0.0)
                if rows > 0:
                    nc.sync.dma_start(ust[:rows, :], u_d[r0 : r0 + rows, :])
                usts.append(ust)
            for dd in range(ND):
                ptf = psT.tile([P, CSUB, P], F32, tag="ptf", name="ptf")
                for s in range(CSUB):
                    nc.tensor.transpose(
                        ptf[:, s, :], usts[s][:, dd * P : (dd + 1) * P], identf[:]
                    )
                nc.scalar.mul(
                    UT[:, dd, ct * C_TILE : (ct + 1) * C_TILE],
                    ptf[:],
                    U_SCALE if mm1_fp8 else 1.0,
                )
            if transpose_mode == "dram" and ct == NCT - 1:
                for dd in range(ND):
                    nc.sync.dma_start(
                        UT[:, dd, :],
                        ubf_d[:, dd * P : (dd + 1) * P],
                        transpose=True,
                    )

        # All operand prep upfront; the transpose PSUM pool closes before the
        # main-loop pools open so its banks are reused for psS.
        for b in range(B_loc):
            prep_k(b)
        for ct in range(NCT):
            prep_u(ct)
        prep_ctx.close()

        psS = ctx.enter_context(tc.tile_pool(name="psS", bufs=2, space="PSUM"))
        psV = ctx.enter_context(tc.tile_pool(name="psV", bufs=1, space="PSUM"))

        for ct in range(NCT):
            for b in range(B_loc):
                # One PSUM bank per c-subtile, separate tags so each bank is
                # released to the next iteration as soon as its own epilogue
                # drain finishes (instead of gating on the whole group).
                pv = [
                    psV.tile([P, 512], F32, tag=f"pv{j}", name=f"pv{j}")
                    for j in range(CSUB)
                ]
                for np_ in range(NL // 2):
                    # S^T tiles for two l-chunks share one psS tile so a single
                    # wide activation (1024 cols) amortizes ACT fixed costs.
                    ps = psS.tile([P, 2, C_TILE], F32, tag="ps", name="ps")
                    for h in range(2):
                        n = 2 * np_ + h
                        if mm1_fp8:
                            # DoubleRow: both 128-deep d-halves contracted by
                            # one matmul (2 fp8 weights/cell), [K,2,N] operands.
                            nc.tensor.matmul(
                                ps[:, h, :],
                                KT[b][:, :, n * P : (n + 1) * P],
                                UT[:, :, ct * C_TILE : (ct + 1) * C_TILE],
                                start=True,
                                stop=True,
                                perf_mode=mybir.MatmulPerfMode.DoubleRow,
                            )
                        else:
                            for dd in range(ND):
                                nc.tensor.matmul(
                                    ps[:, h, :],
                                    KT[b][:, dd, n * P : (n + 1) * P],
                                    UT[:, dd, ct * C_TILE : (ct + 1) * C_TILE],
                                    start=(dd == 0),
                                    stop=(dd == ND - 1),
                                )
                    et = expp.tile([P, 2, C_TILE], BF16, tag="et", name="et")
                    nc.scalar.activation(
                        et[:],
                        ps[:],
                        mybir.ActivationFunctionType.Exp,
                        bias=zbias[:],
                        scale=scale,
                    )
                    for h in range(2):
                        n = 2 * np_ + h
                        for j in range(CSUB):
                            nc.tensor.matmul(
                                pv[j][:, 0 : D + 1],
                                et[:, h, j * P : (j + 1) * P],
                                KA[b][:, n, :],
                                start=(n == 0),
                                stop=(n == NL - 1),
                            )
                for j in range(CSUB):
                    c0 = ct * C_TILE + j * P
                    rows = min(P, C - c0)
                    if rows <= 0:
                        continue
                    rec = stage.tile([P, 1], F32, tag="rec", name="rec")
                    nc.vector.reciprocal(rec[:], pv[j][:, D : D + 1])
                    vo = outp.tile([P, D], F32, tag="vo", name="vo")
                    nc.vector.tensor_scalar_mul(vo[:], pv[j][:, 0:D], rec[:])
                    nc.sync.dma_start(out_d[b, c0 : c0 + rows, :], vo[:rows, :])

    nc.compile()
    return nc


_NC_CACHE = {}


def _get_nc(**kw):
    key = tuple(sorted(kw.items()))
    if key not in _NC_CACHE:
        _NC_CACHE[key] = _build_nc(**kw)
    return _NC_CACHE[key]


def kernel_with_results(keys, U_weight, trace=False, **build_kw):
    """Run on 8 NeuronCores; returns (full_output, BassKernelResults)."""
    from concourse.bass_utils import run_bass_kernel_spmd

    keys = np.ascontiguousarray(np.asarray(keys, dtype=np.float32))
    U_weight = np.ascontiguousarray(np.asarray(U_weight, dtype=np.float32))
    B = keys.shape[0]
    assert B % N_CORES == 0
    b_loc = B // N_CORES

    nc = _get_nc(B_loc=b_loc, L=keys.shape[1], C=U_weight.shape[0], D=keys.shape[2],
                 **build_kw)
    in_maps = [
        {
            "keys": np.ascontiguousarray(keys[i * b_loc : (i + 1) * b_loc]),
            "U_weight": U_weight,
        }
        for i in range(N_CORES)
    ]
    res = run_bass_kernel_spmd(
        nc, in_maps, core_ids=list(range(N_CORES)), trace=trace
    )
    out = np.concatenate([r["out"] for r in res.results], axis=0)
    return out, res


def kernel(keys, U_weight):
    out, _ = kernel_with_results(keys, U_weight)
    return out
